# revision 29
# baseline (speedup 1.0000x reference)
"""Trainium2 Bass kernel for BlankEmbedding (embedding lookup + blank shift-accumulate).

Reference semantics:
    out = emb[x]                                    # [B, S, D] gather
    preblank[p] = (x[p+1]==BLANK) & (x[p]!=BLANK)   (per row; zero-padded shifts)
    out[p+k] += preblank[p] * emb[x[p]]  for k in 1..3

Data-parallel over the 16384 flattened tokens, 2048 per core. The device
gathers int8-quantized rows (global absmax/127 scale; ~7.8e-3 rel err vs
the 2e-2 budget) and stores them unmodified; the host applies the scale
and inverts the placement permutation while unsharding. Blank fixups are
recomputed on-device in int16 and placed by the host.

Two-stream descgen (all behavior HW-measured this session): SWDGE
descriptor generation is the bottleneck. The Pool complex offers
- the engine-synchronous stream (~7ns/row, blocks the engine), and
- a background worker (InstDMAGatherAnt on queues 1-3: ~70ns dispatch,
  ~3ns/row) that needs a ~9us engine-blocking mlp ucode library load and
  only accepts <=768-idx instructions asynchronously in non-first
  positions (1024-idx ones fall back to engine-sync; >1024 wedges the
  Q7). The first ant instruction of a program always engine-syncs.
So: load the library first (the engine is blocked anyway), absorb the
first-sync rule with a 16-idx dummy, dispatch two 768-idx gathers onto
the worker, and run the remaining ~640-idx gather plus the INDIRECT1D
fixup on the engine stream concurrently. Both streams finish within
~1us of each other.

Ant indices are int16, so the table ships as two halves with a zero row
each: emb8a[0]=0, emb8a[1+r]=row r (r<32767); emb8b[r-32767]=row r
(r>=32767), emb8b[17490]=0; the full int32-indexed emb8f serves the
fixup gather (slot k's xt/s1/s2 at partitions k/32+k/64+k, 32-aligned
for the DMA start-partition rule). Ant list position i lands at
tile[i%128, i//128]; idx tiles are int16 [128, n/16], idx j at
[j%16, j//16], replicated 8x across partitions. Per-core half counts
vary; capacities ka/kb are maxed over cores (SPMD: one program) and
padded with each half's zero row.
"""

import numpy as np

VOCAB = 50257
ZROW = VOCAB                 # zero row index in the full table
DIM = 1024
BLANK = 100
N_BLANKS = 3
B, S = 4, 4096
N_CORES = 8
TOK = B * S
TPC = TOK // N_CORES         # 2048 tokens per core
P = 128
ASPLIT = 32767               # values < ASPLIT -> half A
NB_ROWS = VOCAB - ASPLIT + 1  # 17491: B rows + trailing zero row
BZERO = NB_ROWS - 1
KFIX = 16
ACHUNK = 768                 # largest reliably-async dma_gather size

_CACHE = {}


def _build_nc(ka, kb):
    from concourse import bacc, mybir, tile, library_config
    import concourse.bass as bass

    ca, cb = ka // P, kb // P
    wa, wb = ka // 16, kb // 16
    a0 = min(ka, ACHUNK)          # worker chunk
    a1 = ka - a0                  # engine-stream chunk

    nc = bacc.Bacc(
        "TRN2", target_bir_lowering=False, debug=False, num_devices=1,
        num_swdge_queues=4,
    )
    i8 = mybir.dt.int8
    i16 = mybir.dt.int16
    i32 = mybir.dt.int32

    ix_dram = nc.dram_tensor("ix", [P, wa + wb], i16, kind="ExternalInput")
    fix_dram = nc.dram_tensor("fix", [P, 1], i32, kind="ExternalInput")
    emb8a = nc.dram_tensor("emb8a", [ASPLIT + 1, DIM], i8,
                           kind="ExternalInput")
    emb8b = nc.dram_tensor("emb8b", [NB_ROWS, DIM], i8, kind="ExternalInput")
    emb8f = nc.dram_tensor("emb8f", [VOCAB + 1, DIM], i8,
                           kind="ExternalInput")
    out = nc.dram_tensor("out", [P, (ca + cb) * DIM], i8,
                         kind="ExternalOutput")
    fixout = nc.dram_tensor("fixout", [KFIX, DIM], i16, kind="ExternalOutput")

    with tile.TileContext(nc) as tc:
        with tc.tile_pool(name="sbuf", bufs=1) as pool:
            ixt = pool.tile([P, wa + wb], i16)
            fix_sb = pool.tile([P, 1], i32)
            nc.sync.dma_start(out=ixt[:], in_=ix_dram[:])
            nc.scalar.dma_start(out=fix_sb[:], in_=fix_dram[:])

            big = pool.tile([P, (ca + cb) * DIM], i8)
            big3 = big[:].rearrange("p (c d) -> p c d", c=ca + cb, d=DIM)
            dumb = pool.tile([P, DIM], i8)
            dumb3 = dumb[:].rearrange("p (c d) -> p c d", c=1, d=DIM)

            nc.gpsimd.load_library(library_config.mlp)
            # 16-idx dummy absorbs the first-ant-syncs rule (~1.5us)
            nc.gpsimd.dma_gather(dumb3[:, :, :], emb8a[:], ixt[:, 0:1],
                                 16, 16, DIM, elem_step=DIM, queue_num=1)
            # worker stream: two <=768-idx gathers on fresh queues
            nc.gpsimd.dma_gather(big3[:, 0 : a0 // P, :], emb8a[:],
                                 ixt[:, 0 : a0 // 16],
                                 a0, a0, DIM, elem_step=DIM, queue_num=2)
            nc.gpsimd.dma_gather(big3[:, ca : ca + cb, :], emb8b[:],
                                 ixt[:, wa : wa + wb],
                                 kb, kb, DIM, elem_step=DIM, queue_num=3)
            # engine stream: the A remainder (queue reuse -> engine-sync,
            # which is exactly the concurrency we want) + the fixup gather
            if a1:
                nc.gpsimd.dma_gather(big3[:, a0 // P : ca, :], emb8a[:],
                                     ixt[:, a0 // 16 : wa],
                                     a1, a1, DIM, elem_step=DIM, queue_num=1)
            fx = pool.tile([P, DIM], i8)
            nc.gpsimd.indirect_dma_start(
                out=fx[:80, :], out_offset=None, in_=emb8f[:],
                in_offset=bass.IndirectOffsetOnAxis(
                    ap=fix_sb[:80, 0:1], axis=0
                ),
            )

            # region stores fire as their gathers' DMAs complete; the
            # engine-stream chunk finishes last and stores last
            nc.sync.dma_start(out=out[:, 0 : a0 // P * DIM],
                              in_=big[:, 0 : a0 // P * DIM])
            nc.sync.dma_start(out=out[:, ca * DIM : (ca + cb) * DIM],
                              in_=big[:, ca * DIM : (ca + cb) * DIM])
            if a1:
                nc.sync.dma_start(out=out[:, a0 // P * DIM : ca * DIM],
                                  in_=big[:, a0 // P * DIM : ca * DIM])

            # fixout[k] = emb8f[xt_k] + emb8f[s1_k] + emb8f[s2_k] in int16
            w0 = pool.tile([P, DIM], i16)
            nc.vector.tensor_scalar(
                out=w0[:80, :], in0=fx[:80, :],
                scalar1=1.0, scalar2=None, op0=mybir.AluOpType.mult,
            )
            g1 = pool.tile([P, DIM], i16)
            g2 = pool.tile([P, DIM], i16)
            nc.scalar.dma_start(out=g1[0:KFIX, :], in_=w0[32 : 32 + KFIX, :])
            nc.scalar.dma_start(out=g2[0:KFIX, :], in_=w0[64 : 64 + KFIX, :])
            nc.vector.tensor_tensor(
                out=g1[0:KFIX, :], in0=g1[0:KFIX, :], in1=g2[0:KFIX, :],
                op=mybir.AluOpType.add,
            )
            nc.vector.tensor_tensor(
                out=w0[0:KFIX, :], in0=w0[0:KFIX, :], in1=g1[0:KFIX, :],
                op=mybir.AluOpType.add,
            )
            nc.scalar.dma_start(out=fixout[:], in_=w0[:KFIX, :])

    nc.compile()
    return nc


def get_nc(ka, kb):
    key = (ka, kb)
    if key not in _CACHE:
        _CACHE[key] = _build_nc(ka, kb)
    return _CACHE[key]


def _corrections(x2):
    """Exact reference semantics: list of (global_target_row, src_token)."""
    is_blank = x2 == BLANK
    prev = np.zeros_like(is_blank)
    prev[:, 1:] = is_blank[:, :-1]
    first_blank = is_blank & ~prev
    out = []
    for b, f in np.argwhere(first_blank):
        if f == 0:
            continue  # run at row start: reference shifts in zeros
        p = f - 1
        src_tok = int(x2[b, p])
        for k in range(1, N_BLANKS + 1):
            s = p + k
            if s >= S:
                break
            out.append((b * S + s, src_tok))
    return out


def _round_up(n, m):
    return (n + m - 1) // m * m


def _idx_block(vals, cap, pad):
    """int16 idx layout: idx j at [j%16, j//16], replicated to 128 rows."""
    padded = np.full(cap, pad, dtype=np.int16)
    padded[: len(vals)] = vals
    block = padded.reshape(cap // 16, 16).T
    return np.tile(block, (P // 16, 1))


def shard_inputs(x, emb_table):
    """Returns (in_maps, perms, fix_targets, ka, kb, scale)."""
    x2 = np.asarray(x).astype(np.int64).reshape(B, S)
    flat = x2.reshape(-1).astype(np.int32)
    emb_f = np.asarray(emb_table, dtype=np.float32)
    scale = float(np.abs(emb_f).max()) / 127.0
    emb_i8 = np.clip(np.rint(emb_f / scale), -127, 127).astype(np.int8)
    zrow = np.zeros((1, DIM), dtype=np.int8)
    emb8a = np.ascontiguousarray(np.vstack([zrow, emb_i8[:ASPLIT]]))
    emb8b = np.ascontiguousarray(np.vstack([emb_i8[ASPLIT:], zrow]))
    emb8f = np.ascontiguousarray(np.vstack([emb_i8, zrow]))

    per_tgt = {}
    for tgt, src in _corrections(x2):
        per_tgt.setdefault(tgt, []).append(src)
    assert all(len(v) <= 2 for v in per_tgt.values()), per_tgt

    splits = []
    for c in range(N_CORES):
        t = flat[c * TPC : (c + 1) * TPC]
        in_a = t < ASPLIT
        splits.append((np.nonzero(in_a)[0], np.nonzero(~in_a)[0]))
    ka = _round_up(max(len(oa) for oa, _ in splits), P)
    kb = _round_up(max(len(ob) for _, ob in splits), P)
    assert ka <= 2 * ACHUNK and kb <= ACHUNK, (ka, kb)

    in_maps = []
    perms = []
    fix_targets = []
    for c in range(N_CORES):
        base = c * TPC
        t = flat[base : base + TPC]
        oa, ob = splits[c]

        ix = np.concatenate(
            [
                _idx_block((t[oa] + 1).astype(np.int16), ka, 0),
                _idx_block((t[ob] - ASPLIT).astype(np.int16), kb, BZERO),
            ],
            axis=1,
        )
        perm = np.empty(TPC, dtype=np.int64)
        perm[oa] = np.arange(len(oa))
        perm[ob] = ka + np.arange(len(ob))
        perms.append(perm)

        fix = np.full((P, 1), ZROW, dtype=np.int32)
        mine = {t_: v for t_, v in per_tgt.items() if base <= t_ < base + TPC}
        assert len(mine) <= KFIX, "fixup slot overflow"
        targets = {}
        for slot, (tgt, srcs) in enumerate(mine.items()):
            fix[slot, 0] = flat[tgt]
            fix[32 + slot, 0] = srcs[0]
            if len(srcs) > 1:
                fix[64 + slot, 0] = srcs[1]
            targets[slot] = tgt - base
        fix_targets.append(targets)
        in_maps.append(
            {"ix": ix, "fix": fix,
             "emb8a": emb8a, "emb8b": emb8b, "emb8f": emb8f}
        )
    return in_maps, perms, fix_targets, ka, kb, scale


def assemble_output(results, perms, fix_targets, ncols, scale):
    parts = []
    for c in range(N_CORES):
        raw = results[c]["out"].reshape(P, ncols, DIM)
        slots = raw.transpose(1, 0, 2).reshape(-1, DIM)
        part = slots[perms[c]].astype(np.float32) * scale
        targets = fix_targets[c]
        if targets:
            fo = results[c]["fixout"]
            for slot, loc in targets.items():
                part[loc] = fo[slot].astype(np.float32) * scale
        parts.append(part)
    return np.concatenate(parts, axis=0).reshape(B, S, DIM)


def kernel(x, emb_table):
    from concourse.bass_utils import run_bass_kernel_spmd

    in_maps, perms, fix_targets, ka, kb, scale = shard_inputs(x, emb_table)
    nc = get_nc(ka, kb)
    res = run_bass_kernel_spmd(nc, in_maps, core_ids=list(range(N_CORES)))
    return assemble_output(
        res.results, perms, fix_targets, (ka + kb) // P, scale
    )


# revision 30
# speedup vs baseline: 1.0522x; 1.0522x over previous
"""Trainium2 Bass kernel for BlankEmbedding (embedding lookup + blank shift-accumulate).

Reference semantics:
    out = emb[x]                                    # [B, S, D] gather
    preblank[p] = (x[p+1]==BLANK) & (x[p]!=BLANK)   (per row; zero-padded shifts)
    out[p+k] += preblank[p] * emb[x[p]]  for k in 1..3

Strategy: data-parallel over the 16384 flattened tokens, 2048 per core.
The device gathers int8-quantized rows (global absmax/127 scale; ~7.8e-3
rel err vs the 2e-2 budget) and stores them unmodified; the host applies
the scale while unsharding. Sparse blank fixups (P(blank)=1/50257) are
recomputed on-device in int16 and placed by the host.

- Gathers run on the SWDGE indirect-DMA path: descgen is the bottleneck
  (~1.1us per 128-row instruction, engine-serial; measured that neither
  multiple SWDGE queues nor InstDMAGatherAnt beat it once its ~9us mlp
  ucode library load is accounted). Layout ix[p, j] = token 16p + j, so
  each partition holds 16 consecutive tokens and each store descriptor
  is contiguous in DRAM.
- int8 end-to-end halves both the random-row reads (1KB rows) and the
  store traffic vs the bf16 variant, and removes the DVE dequant stage.
- The two fixup gathers sit right after the first main gather so their
  adds + fixout store complete under the main chain instead of tailing
  it. Unused fixup slots read the appended zero row (index VOCAB).
"""

import numpy as np

VOCAB = 50257
ZROW = VOCAB                 # appended all-zeros table row (no-op addend)
DIM = 1024
BLANK = 100
N_BLANKS = 3
B, S = 4, 4096
N_CORES = 8
TOK = B * S                  # 16384 flattened tokens
TPC = TOK // N_CORES         # 2048 tokens per core
P = 128                      # SBUF partitions
NJ = TPC // P                # 16 tokens per partition

_CACHE = {}


KFIX = 16


def _build_nc():
    from concourse import bacc, mybir, tile
    import concourse.bass as bass

    nc = bacc.Bacc(
        "TRN2", target_bir_lowering=False, debug=False, num_devices=1
    )
    i8 = mybir.dt.int8
    i16 = mybir.dt.int16
    i32 = mybir.dt.int32

    ix_dram = nc.dram_tensor("ix", [P, NJ], i32, kind="ExternalInput")
    emb8 = nc.dram_tensor("emb8", [VOCAB + 1, DIM], i8, kind="ExternalInput")
    fix_dram = nc.dram_tensor("fix", [P, 1], i32, kind="ExternalInput")
    out = nc.dram_tensor("out", [TPC, DIM], i8, kind="ExternalOutput")
    fixout = nc.dram_tensor("fixout", [KFIX, DIM], i16, kind="ExternalOutput")

    with tile.TileContext(nc) as tc:
        with tc.tile_pool(name="sbuf", bufs=1) as pool:
            ix_all = pool.tile([P, NJ], i32)
            fix_sb = pool.tile([P, 1], i32)
            # ix on gpsimd's own SWDGE queue: ~1us descgen right after the
            # entry barrier beats the cross-engine HWDGE latency (~2.9us)
            nc.gpsimd.dma_start(out=ix_all[:], in_=ix_dram[:])
            nc.scalar.dma_start(out=fix_sb[:], in_=fix_dram[:])

            g8 = pool.tile([P, NJ * DIM], i8)
            out3 = out[:].rearrange("(p j) d -> p j d", p=P, j=NJ)

            def main_gather(j):
                nc.gpsimd.indirect_dma_start(
                    out=g8[:, j * DIM : (j + 1) * DIM],
                    out_offset=None,
                    in_=emb8[:],
                    in_offset=bass.IndirectOffsetOnAxis(
                        ap=ix_all[:, j : j + 1], axis=0
                    ),
                )
                nc.sync.dma_start(
                    out=out3[:, j : j + 1, :],
                    in_=g8[:, j * DIM : (j + 1) * DIM],
                )

            # single fixup gather rides second in the descgen chain: slot k's
            # xt/s1/s2 addends sit at partitions k / 32+k / 64+k (32-aligned
            # groups for the SBUF-copy realignment below); unused slots read
            # the appended zero... ZROW row
            main_gather(0)
            fx = pool.tile([P, DIM], i8)
            nc.gpsimd.indirect_dma_start(
                out=fx[:80, :], out_offset=None, in_=emb8[:],
                in_offset=bass.IndirectOffsetOnAxis(
                    ap=fix_sb[:80, 0:1], axis=0
                ),
            )
            for j in range(1, NJ):
                main_gather(j)

            # fixout[k] = emb8[xt_k] + emb8[s1_k] + emb8[s2_k] in int16
            w0 = pool.tile([P, DIM], i16)
            nc.vector.tensor_scalar(
                out=w0[:80, :], in0=fx[:80, :],
                scalar1=1.0, scalar2=None, op0=mybir.AluOpType.mult,
            )
            g1 = pool.tile([P, DIM], i16)
            g2 = pool.tile([P, DIM], i16)
            nc.scalar.dma_start(out=g1[0:KFIX, :], in_=w0[32 : 32 + KFIX, :])
            nc.scalar.dma_start(out=g2[0:KFIX, :], in_=w0[64 : 64 + KFIX, :])
            nc.vector.tensor_tensor(
                out=g1[0:KFIX, :], in0=g1[0:KFIX, :], in1=g2[0:KFIX, :],
                op=mybir.AluOpType.add,
            )
            nc.vector.tensor_tensor(
                out=w0[0:KFIX, :], in0=w0[0:KFIX, :], in1=g1[0:KFIX, :],
                op=mybir.AluOpType.add,
            )
            nc.scalar.dma_start(out=fixout[:], in_=w0[:KFIX, :])

    nc.compile()
    return nc


def get_nc():
    if "nc" not in _CACHE:
        _CACHE["nc"] = _build_nc()
    return _CACHE["nc"]


def _corrections(x2):
    """Exact reference semantics: list of (global_target_row, src_token)."""
    is_blank = x2 == BLANK
    prev = np.zeros_like(is_blank)
    prev[:, 1:] = is_blank[:, :-1]
    first_blank = is_blank & ~prev
    out = []
    for b, f in np.argwhere(first_blank):
        if f == 0:
            continue  # run at row start: reference shifts in zeros
        p = f - 1
        src_tok = int(x2[b, p])
        for k in range(1, N_BLANKS + 1):
            s = p + k
            if s >= S:
                break
            out.append((b * S + s, src_tok))
    return out


def shard_inputs(x, emb_table):
    """Returns (in_maps, fix_targets, kfix, has2, scale); fix_targets[c]
    maps fixout slot -> core-local target row."""
    x2 = np.asarray(x).astype(np.int64).reshape(B, S)
    flat = x2.reshape(-1).astype(np.int32)
    emb_f = np.asarray(emb_table, dtype=np.float32)
    scale = float(np.abs(emb_f).max()) / 127.0
    emb_i8 = np.vstack(
        [
            np.clip(np.rint(emb_f / scale), -127, 127).astype(np.int8),
            np.zeros((1, DIM), dtype=np.int8),
        ]
    )

    # per-target slots: tgt -> up to 2 src tokens (two blank runs can land
    # on one target only at distance 2; adjacent first-blanks are impossible)
    per_tgt = {}
    for tgt, src in _corrections(x2):
        per_tgt.setdefault(tgt, []).append(src)
    assert all(len(v) <= 2 for v in per_tgt.values()), per_tgt

    in_maps = []
    fix_targets = []
    for c in range(N_CORES):
        base = c * TPC
        ix = np.ascontiguousarray(flat[base : base + TPC].reshape(P, NJ))

        # slot k: xt at partition k, s1 at 32+k, s2 at 64+k; ZROW elsewhere
        fix = np.full((P, 1), ZROW, dtype=np.int32)
        mine = {t: v for t, v in per_tgt.items() if base <= t < base + TPC}
        assert len(mine) <= KFIX, "fixup slot overflow"
        targets = {}
        for slot, (t, srcs) in enumerate(mine.items()):
            fix[slot, 0] = flat[t]
            fix[32 + slot, 0] = srcs[0]
            if len(srcs) > 1:
                fix[64 + slot, 0] = srcs[1]
            targets[slot] = t - base
        fix_targets.append(targets)
        in_maps.append({"ix": ix, "emb8": emb_i8, "fix": fix})
    return in_maps, fix_targets, scale


def assemble_output(results, fix_targets, scale):
    parts = []
    for c in range(N_CORES):
        part = results[c]["out"].astype(np.float32) * scale
        targets = fix_targets[c]
        if targets:
            fo = results[c]["fixout"]
            for slot, loc in targets.items():
                part[loc] = fo[slot].astype(np.float32) * scale
        parts.append(part)
    return np.concatenate(parts, axis=0).reshape(B, S, DIM)


def kernel(x, emb_table):
    from concourse.bass_utils import run_bass_kernel_spmd

    in_maps, fix_targets, scale = shard_inputs(x, emb_table)
    nc = get_nc()
    res = run_bass_kernel_spmd(nc, in_maps, core_ids=list(range(N_CORES)))
    return assemble_output(res.results, fix_targets, scale)
